# revision 14
# baseline (speedup 1.0000x reference)
"""v7: custom-DVE fused max-scan CVLoss kernel.

Per half-row (P=128 rows of F=16000), the CV stats come from:
  M_j = position of last spike <= j  (prefix max of j*x_j), and
  sum-of-ages S = sum_j (j - M_j) which yields
    sum d_i^2 = 2S - f(f-1) - (F-l)(F-l+1) + (l - f).

A custom DVE op  SPIKE_M_SCAN_SUM:
    out = scan(MAX, Src0*(Src1+C1), init=C0);  accum_out = sum(out)
computes M, its chunk carry (out[:, -1]), and sum(M) in ONE pass at
~1 cyc/elem (the stock tensor_tensor_scan runs at 2 cyc/elem and would
need a separate multiply and reduction). Src0 = raw f32 spikes (no cast
needed), Src1 = a shared 1000-wide local iota, C1 = chunk offset,
C0 = carry from the previous chunk (per-partition AP).

Engine budget per core: DMA 23.4us (8x 1MB chunk loads) > DVE ~20us
(16 fused scans + f-extraction) > ACT ~17us (8 spike-count passes).
DMA-bound.

Host: sum(ages) = sum(positions) - sum(M) per 1000-slice (exact: each
sum(M) partial stays < 2^24 in fp32), f - 1 = #(M==0) over the first
1000 columns (host falls back to argmax on its own copy for rows with
no spike there), l = final M. Merge halves -> per-neuron CV -> loss.
"""

import numpy as np

B, T, N = 16, 2000, 512
L = B * T
NCORES = 8
NPC = N // NCORES
HALVES = 2
P = NPC * HALVES
F = L // HALVES
# DMA/ACT chunk widths (uint8 input; small first chunk starts the scan
# train early) and DVE scan slice widths per chunk. Every slice (offset o,
# width w) keeps w*(o+w) <= 2^24 so the fp32 sum(M) accumulator is exact.
DMA_CHUNKS = (1000, 3000, 4000, 4000, 4000)
SLICE_PLAN = ((1000,), (3000,), (2000, 2000),
              (1000, 1000, 1000, 1000), (1000, 1000, 1000, 1000))
WS0 = SLICE_PLAN[0][0]     # first-slice width (f detection range)
NCH = len(DMA_CHUNKS)
SLICES = [w for ws in SLICE_PLAN for w in ws]
SLICE_OFF = []
_o = 0
for _w in SLICES:
    SLICE_OFF.append(_o)
    _o += _w
assert _o == F
for _w, _oo in zip(SLICES, SLICE_OFF):
    assert _w * (_oo + _w) <= 1 << 24
NSL = len(SLICES)
IOTA_W = max(SLICES)
# acc columns: [0:NCH]=k_c ; [NCH:NCH+NSL]=sum(M) per slice ;
# [NCH+NSL]=#(M>0) over first slice ; [NCH+NSL+1]=l
NACC = NCH + NSL + 2

_BUILD_CACHE = {}


def register_op():
    """Register the fused scan op via the documented custom-DVE extension
    point (concourse dve_ops registry); idempotent."""
    from operator import add
    from concourse.dve_ops import DveOp, OPS, CUSTOM_DVE_SPECS, \
        _SUB_OPCODE_FOR_NAME, _CUSTOM_DVE_ROW_BASE
    from concourse.dve_spec import Spec, Src0, Src1, C0, C1, AluOp, scan, \
        lower
    from concourse.dve_uop import DveOpSpec
    from concourse.dve_table_gen import dve_ver_for

    name = "SPIKE_M_SCAN_SUM"
    if name in _SUB_OPCODE_FOR_NAME:
        return next(op for op in OPS if op.name == name)

    def _ref(in0, in1, s0, s1, imm2):
        v = in0.astype(np.float32) * (in1.astype(np.float32) + s1)
        m = np.maximum.accumulate(v, axis=-1)
        m = np.maximum(m, np.asarray(s0, dtype=np.float32).reshape(-1, 1))
        return m, m.astype(np.float32).sum(axis=-1, keepdims=True)

    spec = Spec(
        body=scan(AluOp.MAX, Src0 * (Src1 + C1), init=C0),
        accum=add,
        reference=_ref,
    )
    row = _CUSTOM_DVE_ROW_BASE + len(OPS)
    _SUB_OPCODE_FOR_NAME[name] = row
    ver = dve_ver_for("TRN2")
    uops = lower(spec, ver=ver)
    sha = DveOpSpec(name=name, opcode=row, uops=uops, rd1_en=True).sha(ver)
    op = DveOp(name, spec, subdim=False, uops_sha={ver: sha})
    OPS.append(op)
    CUSTOM_DVE_SPECS[name] = spec
    return op


def build_bass(P_=P):
    import concourse.bass as bass
    from concourse import bacc
    import concourse.mybir as mybir
    from concourse import tile

    op = register_op()
    Alu = mybir.AluOpType
    AF = mybir.ActivationFunctionType
    f32 = mybir.dt.float32
    i16 = mybir.dt.int16
    u8 = mybir.dt.uint8

    nc = bacc.Bacc(trn_type="TRN2")
    x = nc.dram_tensor("x", (P_, F), u8, kind="ExternalInput")
    io = nc.dram_tensor("io", (P_, IOTA_W), i16, kind="ExternalInput")
    acc = nc.dram_tensor("acc", (P_, NACC), f32, kind="ExternalOutput")

    with tile.TileContext(nc) as tc:
        with tc.tile_pool(name="persist", bufs=1) as pp, \
             tc.tile_pool(name="xin", bufs=5) as xp, \
             tc.tile_pool(name="work", bufs=3) as wp:
            iota = pp.tile([P_, IOTA_W], i16)
            nc.scalar.dma_start(out=iota[:], in_=io[:])
            accs = pp.tile([P_, NACC], f32)

            m_tiles = []
            chunk_off = []
            _co = 0
            for w in DMA_CHUNKS:
                chunk_off.append(_co)
                _co += w

            def load(c):
                w = DMA_CHUNKS[c]
                xc = xp.tile([P_, w], u8, tag=f"xc{c}", name=f"xc{c}")
                nc.sync.dma_start(out=xc[:],
                                  in_=x[:, chunk_off[c]:chunk_off[c] + w])
                return xc

            def kpass(c, xc):
                w = DMA_CHUNKS[c]
                scr = wp.tile([P_, w], i16, tag="scr", name=f"scr{c}")
                nc.scalar.activation(
                    out=scr[:], in_=xc[:], func=AF.Copy,
                    accum_out=accs[:, c:c + 1])

            def scans(c, xc):
                s0_idx = sum(len(SLICE_PLAN[i]) for i in range(c))
                lo = 0
                for h, w in enumerate(SLICE_PLAN[c]):
                    s = s0_idx + h
                    tag = "m0" if s == 0 else "m"
                    m = wp.tile([P_, w], f32, tag=tag, name=f"m{s}")
                    if s == 0:
                        init = 0.0
                    else:
                        mp = m_tiles[s - 1]
                        init = mp[:, mp.shape[1] - 1:mp.shape[1]]
                    nc.vector._custom_dve(
                        op, out=m[:], in0=xc[:, lo:lo + w],
                        in1=iota[:, :w],
                        s0=init, s1=float(SLICE_OFF[s]),
                        accum_out=accs[:, NCH + s:NCH + s + 1])
                    m_tiles.append(m)
                    lo += w

            xc_pend = load(0)
            for c in range(NCH):
                xc_next = load(c + 1) if c + 1 < NCH else None
                scans(c, xc_pend)
                kpass(c, xc_pend)
                if c == 1:
                    # WS0 - (f-1) = #(M > 0) over the first WS0 columns (ACT)
                    eqt = wp.tile([P_, WS0], i16, tag="eqt", name="eqt")
                    nc.scalar.activation(
                        out=eqt[:], in_=m_tiles[0][:], func=AF.Sign,
                        accum_out=accs[:, NCH + NSL:NCH + NSL + 1])
                xc_pend = xc_next
            # all of accs except l is final here: ship it
            nc.sync.dma_start(out=acc[:, :NCH + NSL + 1],
                              in_=accs[:, :NCH + NSL + 1])
            # l = final M (DVE, right behind the last scan in its queue)
            mt = m_tiles[-1]
            nc.vector.tensor_scalar(
                out=accs[:, NCH + NSL + 1:NCH + NSL + 2],
                in0=mt[:, mt.shape[1] - 1:mt.shape[1]],
                scalar1=0.0, scalar2=None, op0=Alu.add)
            nc.sync.dma_start(out=acc[:, NCH + NSL + 1:],
                              in_=accs[:, NCH + NSL + 1:])
    nc.finalize()
    return nc


def get_bass():
    key = (F, DMA_CHUNKS, SLICE_PLAN, P)
    if key not in _BUILD_CACHE:
        _BUILD_CACHE[key] = build_bass()
    return _BUILD_CACHE[key]


def shard_input(output_spikes):
    x = np.asarray(output_spikes, dtype=np.float32)
    maps = []
    for c in range(NCORES):
        xc = x[:, :, c * NPC:(c + 1) * NPC]
        xt = np.ascontiguousarray(
            np.transpose(xc, (2, 0, 1))).reshape(NPC, L).astype(np.uint8)
        io = np.broadcast_to(np.arange(1, IOTA_W + 1, dtype=np.int16),
                             (P, IOTA_W)).copy()
        maps.append({"x": xt.reshape(P, F), "io": io})
    return maps


def finish_host(acc_list, target_cv, in_maps=None, F_=F):
    """Merge per-half-row (k, sum M, f, l) into the scalar loss."""
    target = np.asarray(target_cv, dtype=np.float64)
    # sum of positions per slice: sum_{j=o+1..o+w} j
    wv = np.asarray(SLICES, dtype=np.float64)
    ov = np.asarray(SLICE_OFF, dtype=np.float64)
    pos_sum = wv * ov + wv * (wv + 1) / 2.0
    sq_sum = 0.0
    n_valid = 0
    for ci, acc in enumerate(acc_list):
        a = np.asarray(acc, dtype=np.float64)
        P_ = a.shape[0]
        k_h = np.rint(a[:, 0:NCH].sum(axis=1))
        S_h = (pos_sum[None, :] - a[:, NCH:NCH + NSL]).sum(axis=1)
        f_h = np.rint(WS0 - a[:, NCH + NSL] + 1.0)
        l_h = np.rint(a[:, NCH + NSL + 1])
        n_neu = P_ // 2
        for n in range(n_neu):
            p1, p2 = 2 * n, 2 * n + 1
            stats = []
            for p in (p1, p2):
                kk = k_h[p]
                if kk < 1:
                    continue
                ff = f_h[p]
                if ff > WS0:
                    # first spike beyond the first WS cols: recover on host
                    row = in_maps[ci]["x"][p]
                    ff = float(np.argmax(row > 0) + 1)
                ll = l_h[p]
                s2 = (2.0 * S_h[p] - ff * (ff - 1.0)
                      - (F_ - ll) * (F_ - ll + 1.0) + (ll - ff))
                stats.append((kk, ff, ll, s2, p))
            if not stats:
                continue
            kt = sum(s[0] for s in stats)
            if kt < 3:
                continue
            if len(stats) == 2:
                (k1, f1, l1, s2a, _), (k2, f2, l2, s2b, _) = stats
                d_b = (F_ + f2) - l1
                s2 = s2a + s2b + d_b * d_b
                gf, gl = f1, F_ + l2
            else:
                kk, ff, ll, s2, p = stats[0]
                off = F_ if p == p2 else 0.0
                gf, gl = off + ff, off + ll
            s1 = gl - gf
            mean = s1 / (kt - 1.0)
            var = (s2 - s1 * s1 / (kt - 1.0)) / (kt - 2.0)
            std = np.sqrt(var) if var > 0 else 0.0
            if mean <= 0:
                continue
            cv = std / max(mean, 1e-12)
            d = cv - target[ci * NPC + n]
            sq_sum += d * d
            n_valid += 1
    return np.float32(sq_sum / max(n_valid, 1))


def ensure_ntff_hook(so_path="/opt/axon/libaxon_pjrt.so"):
    """Shim antenv.axon_hooks (absent in this image) so trace=True works.

    Mirrors trn_boot._ntff_profile_via_ctypes: drives NRT profiling via the
    axon PJRT .so's C ABI. Safe no-op if anything is missing.
    """
    import sys
    try:
        import antenv.axon_hooks  # noqa: F401
        return
    except ImportError:
        pass
    try:
        import ctypes
        import contextlib
        import types
        import os

        if not os.path.exists(so_path):
            return
        lib = ctypes.CDLL(so_path)
        if not hasattr(lib, "axon_start_nrt_profile"):
            return
        lib.axon_start_nrt_profile.argtypes = [
            ctypes.POINTER(ctypes.c_int64), ctypes.c_size_t]
        lib.axon_start_nrt_profile.restype = ctypes.c_int64
        lib.axon_stop_nrt_profile.argtypes = [ctypes.c_char_p]
        lib.axon_stop_nrt_profile.restype = ctypes.c_int64

        @contextlib.contextmanager
        def _hook(output_dir, device_ids):
            import jax
            jax.devices()
            if device_ids:
                ids = (ctypes.c_int64 * len(device_ids))(*device_ids)
                rc = lib.axon_start_nrt_profile(ids, len(device_ids))
            else:
                rc = lib.axon_start_nrt_profile(None, 0)
            if rc != 0:
                raise RuntimeError(f"axon_start_nrt_profile rc={rc}")
            try:
                yield
            finally:
                n = lib.axon_stop_nrt_profile(str(output_dir).encode())
                print(f"profile: {n} file(s) written to {output_dir}",
                      file=sys.stderr)

        mod = types.ModuleType("antenv.axon_hooks")
        mod.get_axon_ntff_profile_hook = lambda: _hook
        mod.set_axon_ntff_profile_hook = lambda h: None
        import antenv
        sys.modules["antenv.axon_hooks"] = mod
        antenv.axon_hooks = mod
    except Exception:
        pass


def kernel(output_spikes, target_cv):
    from concourse.bass_utils import run_bass_kernel_spmd

    ensure_ntff_hook()
    nc = get_bass()
    in_maps = shard_input(output_spikes)
    res = run_bass_kernel_spmd(nc, in_maps, core_ids=list(range(NCORES)))
    acc_list = [res.results[c]["acc"] for c in range(NCORES)]
    return finish_host(acc_list, target_cv, in_maps=in_maps)
